# revision 50
# baseline (speedup 1.0000x reference)
"""Trainium2 Bass kernel for nn_AttentionLayer (B=4, L=S=2048, D=1024, H=16).

Sharding: 8 cores = (batch b in 0..3) x (head-group g in 0..1); each core
handles one batch and 8 heads (512 of the 1024 q/k/v/o channels).

v3 schedule.  The ACT engine (exp, ~166us) bounds the attention phase and
the tensor engine (~225us of bf16 matmuls) bounds the kernel, so:
  - dense phase: v/q/k projections of the first half (l,s < 1024) only --
    just enough to start attention chunk 0;
  - attention chunk 0 hides the second-half v/q/k projections as tensor
    fillers between heads; chunk 1 hides the chunk-0 output projection;
  - tail: chunk-1 output projection only.

Per-core kernel details (all matmuls bf16, fp32 PSUM):
  - scores use 64-partition contraction (per-head e-block at base
    partition poff in {0,64}); k_sb is head-pair packed -> no zero
    padding / memsets.
  - exp via ScalarE only (scale=1/8 folded), [128, <=1024] tiles.
  - O^T = V_ext^T @ P^T with V_ext = [V | 1]; PSUM row E=64 is the
    softmax denominator.  After the AV stop, one Vector copy stages
    O^T+den to SBUF f32, freeing the PSUM bank immediately; the divide
    chain (DMA den row to partition 0, GpSimd partition-broadcast, fast
    reciprocal, multiply) then runs off the tensor critical path.
  - bk dropped (softmax shift-invariance), bv/bo added on host
    (out += Wo @ bv + bo exactly, softmax rows sum to 1).
  - inputs stream as [128, 1024] half-rows (2KB DMA packets), ordered so
    the first v-projection matmul unblocks ~2 tiles into the stream.

Host: shards/transposes/casts inputs, runs SPMD on 8 cores, sums the two
head-group partial outputs per batch, adds Wo@bv + bo.
"""

import numpy as np
import ml_dtypes

B, L, S, D, H, E = 4, 2048, 2048, 1024, 1024 // 64, 64
NCORES = 8
GROUPS = 2                 # head-groups (tensor-parallel dimension)
HC = H // GROUPS           # heads per core = 8
EC = HC * E                # channels per core = 512
CH = 1024                  # attention l-chunk size

_BF16 = ml_dtypes.bfloat16


def build(L=L, S=S, D=D, HC=HC, E=E, CH=CH, debug=False, dump=False):
    import concourse.bass as bass
    import concourse.mybir as mybir
    import concourse.tile as tile
    from concourse import bacc
    from concourse.masks import make_upper_triangular

    f32 = mybir.dt.float32
    bf16 = mybir.dt.bfloat16

    EC = HC * E
    KD = D // 128           # k-tiles over d = 8
    MB = EC // 128          # e-blocks (128 wide) per core = 4
    LB = L // 128           # l/s blocks of 128 = 16
    NJ = L // CH            # l-chunks = 2
    SBC = CH // 128         # s-blocks per l-chunk = 8
    HPB = 128 // E          # heads per e-block = 2
    IH = 1024               # input half width
    NH = S // IH            # input halves = 2
    PQ = 512                # projection group output width
    NPQ = S // PQ           # projection quarters = 4

    nc = bacc.Bacc(None, target_bir_lowering=False, debug=debug)

    qT = nc.dram_tensor("qT", [D, L], bf16, kind="ExternalInput")
    kT = nc.dram_tensor("kT", [D, S], bf16, kind="ExternalInput")
    vT = nc.dram_tensor("vT", [D, S], bf16, kind="ExternalInput")
    wq = nc.dram_tensor("wq", [D, EC], bf16, kind="ExternalInput")
    wk = nc.dram_tensor("wk", [D, EC], bf16, kind="ExternalInput")
    wv = nc.dram_tensor("wv", [D, EC], bf16, kind="ExternalInput")
    wo = nc.dram_tensor("wo", [EC, D], bf16, kind="ExternalInput")
    bq = nc.dram_tensor("bq", [128, MB], f32, kind="ExternalInput")
    out = nc.dram_tensor("out", [L, D], bf16, kind="ExternalOutput")

    scale = 1.0 / float(np.sqrt(E))

    with tile.TileContext(nc) as tc:
        with (
            tc.tile_pool(name="persist", bufs=1) as pp,
            tc.tile_pool(name="weights", bufs=1) as wp,
            tc.tile_pool(name="inq", bufs=12) as ipq,
            tc.tile_pool(name="ink", bufs=11) as ipk,
            tc.tile_pool(name="inv", bufs=12) as ipv,
            tc.tile_pool(name="work", bufs=5) as kp,
            tc.tile_pool(name="divp", bufs=2) as dp,
            tc.tile_pool(name="psS", bufs=3, space="PSUM") as psS,
            tc.tile_pool(name="psO", bufs=2, space="PSUM") as psO,
        ):
            # ---- persistent SBUF tensors ----
            q_sb = pp.tile([128, MB, L], bf16, tag="q_sb")
            k_sb = pp.tile([128, MB, S], bf16, tag="k_sb")   # head-pair packed
            v_sb = pp.tile([128, S // 128, HC, E + 1], bf16, tag="v_sb")
            o_sb = pp.tile([128, MB, L], bf16, tag="o_sb")
            tri = pp.tile([128, 128], bf16, tag="tri")
            bq_t = pp.tile([128, MB], f32, tag="bq_t")

            wq_sb = wp.tile([128, KD, EC], bf16, tag="wq")
            wk_sb = wp.tile([128, KD, EC], bf16, tag="wk")
            wv_sb = wp.tile([128, KD, EC], bf16, tag="wv")
            wo_sb = wp.tile([128, MB, D], bf16, tag="wo")

            nc.sync.dma_start(bq_t[:], bq[:])
            make_upper_triangular(nc, tri[:, :], val=1.0, diag=True)
            # ones column for the softmax denominator (PSUM partition E=64)
            nc.vector.memset(v_sb[:, :, :, E : E + 1], 1.0)

            # ---- DMA emitters (input halves, 2KB packets) ----
            in_tiles = {}

            def _ld(dram, pool, k, hh):
                tq = pool.tile([128, IH], bf16, tag="t",
                               name=f"in_{dram.name}_{k}_{hh}")
                nc.sync.dma_start(
                    tq[:], dram[128 * k : 128 * (k + 1), IH * hh : IH * (hh + 1)]
                )
                in_tiles[dram.name, k, hh] = tq

            # ---- projection group emitters (one PSUM lifetime each) ----
            def v_group(sb):
                # v[s-block sb, h, e] = values @ Wv.T  (natural layout)
                hh = (128 * sb) // IH
                ps = psS.tile([128, EC], f32, tag="sps", name=f"vp{sb}")
                for k in range(KD):
                    nc.tensor.matmul(
                        ps[:, :],
                        in_tiles["vT", k, hh][
                            :, 128 * sb - IH * hh : 128 * (sb + 1) - IH * hh
                        ],
                        wv_sb[:, k, :],
                        start=(k == 0),
                        stop=(k == KD - 1),
                    )
                nc.vector.tensor_copy(
                    v_sb[:, sb, :, 0:E],
                    ps[:, :].rearrange("p (h e) -> p h e", h=HC),
                )

            def q_group(m, n):
                # q^T[e-block m, l-quarter n] + bias bq
                hh = (PQ * n) // IH
                ps = psS.tile([128, PQ], f32, tag="sps", name=f"qp{m}_{n}")
                for k in range(KD):
                    nc.tensor.matmul(
                        ps[:, :],
                        wq_sb[:, k, 128 * m : 128 * (m + 1)],
                        in_tiles["qT", k, hh][
                            :, PQ * n - IH * hh : PQ * (n + 1) - IH * hh
                        ],
                        start=(k == 0),
                        stop=(k == KD - 1),
                    )
                nc.vector.tensor_scalar_add(
                    q_sb[:, m, PQ * n : PQ * (n + 1)],
                    ps[:, :],
                    bq_t[:, m : m + 1],
                )

            def k_group(m, n):
                # k^T[e-block m, s-quarter n]; head-pair packed, no bias
                hh = (PQ * n) // IH
                ps = psS.tile([128, PQ], f32, tag="sps", name=f"kp{m}_{n}")
                for k in range(KD):
                    nc.tensor.matmul(
                        ps[:, :],
                        wk_sb[:, k, 128 * m : 128 * (m + 1)],
                        in_tiles["kT", k, hh][
                            :, PQ * n - IH * hh : PQ * (n + 1) - IH * hh
                        ],
                        start=(k == 0),
                        stop=(k == KD - 1),
                    )
                nc.vector.tensor_copy(
                    k_sb[:, m, PQ * n : PQ * (n + 1)], ps[:, :]
                )

            def o_group(lb, on_act=False):
                # out[l-block lb, :] = sum_e O^T[e, lb] @ Wo^T[e, :]
                # full-D groups: one 2-bank PSUM slab (same slab as scores),
                # stationary loaded once per e-block for both 512-col spans
                ps = psS.tile([128, D], f32, tag="sps", name=f"op{lb}")
                for k in range(MB):
                    for c in (0, 512):
                        nc.tensor.matmul(
                            ps[:, c : c + 512],
                            o_sb[:, k, 128 * lb : 128 * (lb + 1)],
                            wo_sb[:, k, c : c + 512],
                            start=(k == 0),
                            stop=(k == MB - 1),
                        )
                ot = kp.tile([128, D], bf16, tag="ot", bufs=2,
                             name=f"ot{lb}")
                if on_act:  # tail: exp is done, ACT is idle
                    nc.scalar.activation(ot[:, :], ps[:, :],
                                         mybir.ActivationFunctionType.Copy)
                else:
                    nc.vector.tensor_copy(ot[:, :], ps[:, :])
                nc.sync.dma_start(
                    out[128 * lb : 128 * (lb + 1), :],
                    ot[:, :],
                )

            # ---- attention head-chunk emitter ----
            # Blocks [i_lo, i_hi) of chunk j.  With partial_out, the raw
            # accumulator is staged to SBUF (bf16) and no divide happens;
            # with add_partial, a previously staged partial is added back
            # before the divide.
            def attn_head(h, j, i_lo=0, i_hi=None, partial_out=None,
                          add_partial=None):
                m = h // HPB
                poff = (h % HPB) * E
                nsb = SBC * (j + 1)
                if i_hi is None:
                    i_hi = nsb
                sps = [None] * i_hi
                pt = [None] * i_hi
                # two 1-bank accumulators (spans 0:512 / 512:1024): span a
                # finishes 4 s-blocks early, so its staging overlaps the
                # remaining AVs and the next head's span-a starts sooner
                ops_a = psO.tile([128, 512], f32, tag="ops",
                                 name=f"opsA{h}_{j}_{i_lo}")
                ops_b = psO.tile([128, 512], f32, tag="ops",
                                 name=f"opsB{h}_{j}_{i_lo}")

                def col0(i):
                    return max(0, 128 * i - CH * j)

                def _spans(c0):
                    # split [c0, CH) at 512-col PSUM bank boundaries
                    sp, c = [], c0
                    while c < CH:
                        n = min(512 - (c % 512), CH - c)
                        sp.append((c, n))
                        c += n
                    return sp

                def emit_scores(i):
                    c0 = col0(i)
                    sps[i] = psS.tile([128, CH], f32, tag="sps",
                                      name=f"sps{h}_{i}_{j}")
                    for (c, n) in _spans(c0):
                        nc.tensor.matmul(
                            sps[i][:, c : c + n],
                            k_sb[poff : poff + E, m, 128 * i : 128 * (i + 1)],
                            q_sb[poff : poff + E, m,
                                 CH * j + c : CH * j + c + n],
                            start=True,
                            stop=True,
                        )
                    pt[i] = kp.tile([128, CH], bf16, tag="p_t",
                                    name=f"pt{h}_{i}_{j}")
                    nc.scalar.activation(
                        pt[i][:, c0:CH],
                        sps[i][:, c0:CH],
                        mybir.ActivationFunctionType.Exp,
                        scale=scale,
                    )
                    if i >= SBC * j:  # diagonal band: causal mask
                        nc.vector.tensor_mul(
                            pt[i][:, c0 : c0 + 128],
                            pt[i][:, c0 : c0 + 128],
                            tri[:, :],
                        )

                o_raw = dp.tile([128, CH], f32, tag="o_raw", bufs=2)
                i_last_a = min(i_hi - 1, (512 + CH * j) // 128 - 1)

                def stage(span):
                    # PSUM -> SBUF per span; chunk 0 stages on the (there)
                    # half-idle ACT engine
                    src_t = ops_a if span == 0 else ops_b
                    dst = o_raw[0 : E + 1, 512 * span : 512 * (span + 1)]
                    if j == 0:
                        nc.scalar.activation(
                            dst, src_t[0 : E + 1, :],
                            mybir.ActivationFunctionType.Copy,
                        )
                    else:
                        nc.vector.tensor_copy(dst, src_t[0 : E + 1, :])

                def emit_av(i):
                    c0 = col0(i)
                    for (c, n) in _spans(c0):
                        # last contributing s-block for this 512-col span
                        be = min(CH, 512 * (c // 512) + 512)
                        i_last = min(i_hi - 1, (be + CH * j) // 128 - 1)
                        tile_o = ops_a if c < 512 else ops_b
                        cc = c % 512
                        nc.tensor.matmul(
                            tile_o[0 : E + 1, cc : cc + n],
                            v_sb[:, i, h, :],
                            pt[i][:, c : c + n],
                            start=(i == i_lo),
                            stop=(i == i_last),
                        )
                    if i == i_last_a:
                        stage(0)

                emit_scores(i_lo)
                if i_lo + 1 < i_hi:
                    emit_scores(i_lo + 1)
                for i in range(i_lo + 2, i_hi):
                    emit_scores(i)
                    emit_av(i - 2)
                if i_lo + 1 < i_hi:
                    emit_av(i_hi - 2)
                emit_av(i_hi - 1)
                stage(1)

                # divide chain, off the tensor critical path; the Vector
                # half is returned as a closure so the caller can defer it
                # behind the next head's mask/copy work
                rs0 = dp.tile([128, CH], f32, tag="rs0", bufs=1)
                nc.sync.dma_start(rs0[0:1, :], o_raw[E : E + 1, :])
                rr = dp.tile([128, CH], f32, tag="rr", bufs=2)
                nc.gpsimd.partition_broadcast(
                    rr[0:E, :], rs0[0:1, :], channels=E
                )

                def finish_chain():
                    nc.vector.reciprocal_approx_fast(rr[0:E, :], rr[0:E, :])
                    if poff == 0:
                        nc.vector.tensor_mul(
                            o_sb[0:E, m, CH * j : CH * (j + 1)],
                            o_raw[0:E, :],
                            rr[0:E, :],
                        )
                    else:
                        o_tmp = dp.tile([128, CH], bf16, tag="o_tmp", bufs=2)
                        nc.vector.tensor_mul(o_tmp[0:E, :], o_raw[0:E, :],
                                             rr[0:E, :])
                        nc.sync.dma_start(
                            o_sb[poff : poff + E, m, CH * j : CH * (j + 1)],
                            o_tmp[0:E, :],
                        )

                return finish_chain

            # ---- startup DMA: weights + first halves, unblocking order ----
            for k in range(KD):
                nc.sync.dma_start(wv_sb[:, k, :], wv[128 * k : 128 * (k + 1), :])
                _ld(vT, ipv, k, 0)
            for k in range(KD):
                nc.sync.dma_start(wq_sb[:, k, :], wq[128 * k : 128 * (k + 1), :])
                _ld(qT, ipq, k, 0)
            for k in range(KD):
                nc.sync.dma_start(wk_sb[:, k, :], wk[128 * k : 128 * (k + 1), :])
                _ld(kT, ipk, k, 0)
            for k in range(MB):
                nc.sync.dma_start(wo_sb[:, k, :], wo[128 * k : 128 * (k + 1), :])

            # ---- dense phase: first-half projections (l,s in [0, 1024)) ----
            for sb in range(SBC):
                v_group(sb)
            for m in range(MB):
                for n in range(2):
                    q_group(m, n)
            # second-half input loads chase the dense phase
            for k in range(KD):
                _ld(vT, ipv, k, 1)
            for m in range(MB):
                for n in range(2):
                    k_group(m, n)
            for k in range(KD):
                _ld(qT, ipq, k, 1)
                _ld(kT, ipk, k, 1)

            # ---- attention chunk 0 + second-half projections as fillers ----
            fillers = []
            for sb in range(SBC, 2 * SBC):
                fillers.append(lambda sb=sb: v_group(sb))
            for m in range(MB):
                for n in range(2, 4):
                    fillers.append(lambda m=m, n=n: q_group(m, n))
                    fillers.append(lambda m=m, n=n: k_group(m, n))
            nf, done = len(fillers), 0
            pending = None
            for h in range(HC):
                fin = attn_head(h, 0)
                if pending is not None:
                    pending()
                pending = fin
                upto = nf * (h + 1) // HC
                while done < upto:
                    fillers[done]()
                    done += 1
            pending()

            # ---- attention chunk 1 + chunk-0 output projection (4 of its
            # 16 groups held back to fill the post-AV divide-chain gap) ----
            fillers = [lambda lb=lb: o_group(lb) for lb in range(SBC)]
            reserve = fillers[-2:]
            fillers = fillers[:-2]
            nf, done = len(fillers), 0
            pending = None
            for h in range(HC):
                fin = attn_head(h, 1)
                if pending is not None:
                    pending()
                pending = fin
                upto = nf * (h + 1) // HC
                while done < upto:
                    fillers[done]()
                    done += 1
            pending()

            # ---- tail: reserved chunk-0 groups, then chunk-1 o-proj ----
            for f in reserve:
                f()
            for lb in range(SBC, 2 * SBC):
                o_group(lb, on_act=True)

            if dump:
                dq = nc.dram_tensor("dq", [128, MB, L], bf16, kind="ExternalOutput")
                dk = nc.dram_tensor("dk", [128, MB, S], bf16, kind="ExternalOutput")
                dv = nc.dram_tensor(
                    "dv", [128, S // 128, HC, E + 1], bf16, kind="ExternalOutput"
                )
                do = nc.dram_tensor("do", [128, MB, L], bf16, kind="ExternalOutput")
                nc.sync.dma_start(dq[:], q_sb[:])
                nc.sync.dma_start(dk[:], k_sb[:])
                nc.sync.dma_start(dv[:], v_sb[:])
                nc.sync.dma_start(do[:], o_sb[:])

    nc.compile()
    return nc


def _prep_inputs(queries, keys, values, Wq, bq, Wk, Wv, Wo):
    """Build the 8 per-core input maps (host-side shard + transpose + cast)."""
    MB = EC // 128
    in_maps = []
    qT = [np.ascontiguousarray(queries[b].T.astype(_BF16)) for b in range(B)]
    kT = [np.ascontiguousarray(keys[b].T.astype(_BF16)) for b in range(B)]
    vT = [np.ascontiguousarray(values[b].T.astype(_BF16)) for b in range(B)]
    wqs, wks, wvs, wos, bqs = [], [], [], [], []
    for g in range(GROUPS):
        sl = slice(g * EC, (g + 1) * EC)
        wqs.append(np.ascontiguousarray(Wq[sl, :].T.astype(_BF16)))
        wks.append(np.ascontiguousarray(Wk[sl, :].T.astype(_BF16)))
        wvs.append(np.ascontiguousarray(Wv[sl, :].T.astype(_BF16)))
        wos.append(np.ascontiguousarray(Wo[:, sl].T.astype(_BF16)))
        bqs.append(
            np.ascontiguousarray(
                bq[sl].astype(np.float32).reshape(MB, 128).T
            )
        )
    for c in range(NCORES):
        b, g = c // GROUPS, c % GROUPS
        in_maps.append(
            {
                "qT": qT[b], "kT": kT[b], "vT": vT[b],
                "wq": wqs[g], "wk": wks[g], "wv": wvs[g],
                "wo": wos[g], "bq": bqs[g],
            }
        )
    return in_maps


_NC_CACHE = {}


def kernel(queries, keys, values, attn_mask, Wq, bq, Wk, bk, Wv, bv, Wo, bo,
           _trace=False):
    from concourse.bass_utils import run_bass_kernel_spmd

    queries = np.asarray(queries, np.float32)
    keys = np.asarray(keys, np.float32)
    values = np.asarray(values, np.float32)
    Wq, Wk, Wv, Wo = (np.asarray(a, np.float32) for a in (Wq, Wk, Wv, Wo))
    bq, bk, bv, bo = (np.asarray(a, np.float32) for a in (bq, bk, bv, bo))

    if "nc" not in _NC_CACHE:
        _NC_CACHE["nc"] = build()
    nc = _NC_CACHE["nc"]

    in_maps = _prep_inputs(queries, keys, values, Wq, bq, Wk, Wv, Wo)
    res = run_bass_kernel_spmd(
        nc, in_maps, core_ids=list(range(NCORES)), trace=_trace
    )
    _NC_CACHE["last_results"] = res

    out = np.zeros((B, L, D), np.float32)
    for c in range(NCORES):
        out[c // GROUPS] += np.asarray(res.results[c]["out"], np.float32)
    # bv exits through the (row-sum-1) softmax as Wo @ bv; bo is direct.
    out += (Wo @ bv + bo)[None, None, :]
    return out


# revision 51
# speedup vs baseline: 1.0151x; 1.0151x over previous
"""Trainium2 Bass kernel for nn_AttentionLayer (B=4, L=S=2048, D=1024, H=16).

Sharding: 8 cores = (batch b in 0..3) x (head-group g in 0..1); each core
handles one batch and 8 heads (512 of the 1024 q/k/v/o channels).

v3 schedule.  The ACT engine (exp, ~166us) bounds the attention phase and
the tensor engine (~225us of bf16 matmuls) bounds the kernel, so:
  - dense phase: v/q/k projections of the first half (l,s < 1024) only --
    just enough to start attention chunk 0;
  - attention chunk 0 hides the second-half v/q/k projections as tensor
    fillers between heads; chunk 1 hides the chunk-0 output projection;
  - tail: chunk-1 output projection only.

Per-core kernel details (all matmuls bf16, fp32 PSUM):
  - scores use 64-partition contraction (per-head e-block at base
    partition poff in {0,64}); k_sb is head-pair packed -> no zero
    padding / memsets.
  - exp via ScalarE only (scale=1/8 folded), [128, <=1024] tiles.
  - O^T = V_ext^T @ P^T with V_ext = [V | 1]; PSUM row E=64 is the
    softmax denominator.  After the AV stop, one Vector copy stages
    O^T+den to SBUF f32, freeing the PSUM bank immediately; the divide
    chain (DMA den row to partition 0, GpSimd partition-broadcast, fast
    reciprocal, multiply) then runs off the tensor critical path.
  - bk dropped (softmax shift-invariance), bv/bo added on host
    (out += Wo @ bv + bo exactly, softmax rows sum to 1).
  - inputs stream as [128, 1024] half-rows (2KB DMA packets), ordered so
    the first v-projection matmul unblocks ~2 tiles into the stream.

Host: shards/transposes/casts inputs, runs SPMD on 8 cores, sums the two
head-group partial outputs per batch, adds Wo@bv + bo.
"""

import numpy as np
import ml_dtypes

B, L, S, D, H, E = 4, 2048, 2048, 1024, 1024 // 64, 64
NCORES = 8
GROUPS = 2                 # head-groups (tensor-parallel dimension)
HC = H // GROUPS           # heads per core = 8
EC = HC * E                # channels per core = 512
CH = 1024                  # attention l-chunk size

_BF16 = ml_dtypes.bfloat16


def build(L=L, S=S, D=D, HC=HC, E=E, CH=CH, debug=False, dump=False):
    import concourse.bass as bass
    import concourse.mybir as mybir
    import concourse.tile as tile
    from concourse import bacc
    from concourse.masks import make_upper_triangular

    f32 = mybir.dt.float32
    bf16 = mybir.dt.bfloat16

    EC = HC * E
    KD = D // 128           # k-tiles over d = 8
    MB = EC // 128          # e-blocks (128 wide) per core = 4
    LB = L // 128           # l/s blocks of 128 = 16
    NJ = L // CH            # l-chunks = 2
    SBC = CH // 128         # s-blocks per l-chunk = 8
    HPB = 128 // E          # heads per e-block = 2
    IH = 1024               # input half width
    NH = S // IH            # input halves = 2
    PQ = 512                # projection group output width
    NPQ = S // PQ           # projection quarters = 4

    nc = bacc.Bacc(None, target_bir_lowering=False, debug=debug)

    qT = nc.dram_tensor("qT", [D, L], bf16, kind="ExternalInput")
    kT = nc.dram_tensor("kT", [D, S], bf16, kind="ExternalInput")
    vT = nc.dram_tensor("vT", [D, S], bf16, kind="ExternalInput")
    wq = nc.dram_tensor("wq", [D, EC], bf16, kind="ExternalInput")
    wk = nc.dram_tensor("wk", [D, EC], bf16, kind="ExternalInput")
    wv = nc.dram_tensor("wv", [D, EC], bf16, kind="ExternalInput")
    wo = nc.dram_tensor("wo", [EC, D], bf16, kind="ExternalInput")
    bq = nc.dram_tensor("bq", [128, MB], f32, kind="ExternalInput")
    out = nc.dram_tensor("out", [L, D], bf16, kind="ExternalOutput")

    scale = 1.0 / float(np.sqrt(E))

    with tile.TileContext(nc) as tc:
        with (
            tc.tile_pool(name="persist", bufs=1) as pp,
            tc.tile_pool(name="weights", bufs=1) as wp,
            tc.tile_pool(name="inq", bufs=12) as ipq,
            tc.tile_pool(name="ink", bufs=11) as ipk,
            tc.tile_pool(name="inv", bufs=12) as ipv,
            tc.tile_pool(name="work", bufs=4) as kp,
            tc.tile_pool(name="divp", bufs=2) as dp,
            tc.tile_pool(name="psS", bufs=3, space="PSUM") as psS,
            tc.tile_pool(name="psO", bufs=1, space="PSUM") as psO,
        ):
            # ---- persistent SBUF tensors ----
            q_sb = pp.tile([128, MB, L], bf16, tag="q_sb")
            k_sb = pp.tile([128, MB, S], bf16, tag="k_sb")   # head-pair packed
            v_sb = pp.tile([128, S // 128, HC, E + 1], bf16, tag="v_sb")
            o_sb = pp.tile([128, MB, L], bf16, tag="o_sb")
            tri = pp.tile([128, 128], bf16, tag="tri")
            bq_t = pp.tile([128, MB], f32, tag="bq_t")

            wq_sb = wp.tile([128, KD, EC], bf16, tag="wq")
            wk_sb = wp.tile([128, KD, EC], bf16, tag="wk")
            wv_sb = wp.tile([128, KD, EC], bf16, tag="wv")
            wo_sb = wp.tile([128, MB, D], bf16, tag="wo")

            nc.sync.dma_start(bq_t[:], bq[:])
            make_upper_triangular(nc, tri[:, :], val=1.0, diag=True)
            # ones column for the softmax denominator (PSUM partition E=64)
            nc.vector.memset(v_sb[:, :, :, E : E + 1], 1.0)

            # ---- DMA emitters (input halves, 2KB packets) ----
            in_tiles = {}

            def _ld(dram, pool, k, hh):
                tq = pool.tile([128, IH], bf16, tag="t",
                               name=f"in_{dram.name}_{k}_{hh}")
                nc.sync.dma_start(
                    tq[:], dram[128 * k : 128 * (k + 1), IH * hh : IH * (hh + 1)]
                )
                in_tiles[dram.name, k, hh] = tq

            # ---- projection group emitters (one PSUM lifetime each) ----
            def v_group(sb):
                # v[s-block sb, h, e] = values @ Wv.T  (natural layout)
                hh = (128 * sb) // IH
                ps = psS.tile([128, EC], f32, tag="sps", name=f"vp{sb}")
                for k in range(KD):
                    nc.tensor.matmul(
                        ps[:, :],
                        in_tiles["vT", k, hh][
                            :, 128 * sb - IH * hh : 128 * (sb + 1) - IH * hh
                        ],
                        wv_sb[:, k, :],
                        start=(k == 0),
                        stop=(k == KD - 1),
                    )
                nc.vector.tensor_copy(
                    v_sb[:, sb, :, 0:E],
                    ps[:, :].rearrange("p (h e) -> p h e", h=HC),
                )

            def q_group(m, n):
                # q^T[e-block m, l-quarter n] + bias bq
                hh = (PQ * n) // IH
                ps = psS.tile([128, PQ], f32, tag="sps", name=f"qp{m}_{n}")
                for k in range(KD):
                    nc.tensor.matmul(
                        ps[:, :],
                        wq_sb[:, k, 128 * m : 128 * (m + 1)],
                        in_tiles["qT", k, hh][
                            :, PQ * n - IH * hh : PQ * (n + 1) - IH * hh
                        ],
                        start=(k == 0),
                        stop=(k == KD - 1),
                    )
                nc.vector.tensor_scalar_add(
                    q_sb[:, m, PQ * n : PQ * (n + 1)],
                    ps[:, :],
                    bq_t[:, m : m + 1],
                )

            def k_group(m, n):
                # k^T[e-block m, s-quarter n]; head-pair packed, no bias
                hh = (PQ * n) // IH
                ps = psS.tile([128, PQ], f32, tag="sps", name=f"kp{m}_{n}")
                for k in range(KD):
                    nc.tensor.matmul(
                        ps[:, :],
                        wk_sb[:, k, 128 * m : 128 * (m + 1)],
                        in_tiles["kT", k, hh][
                            :, PQ * n - IH * hh : PQ * (n + 1) - IH * hh
                        ],
                        start=(k == 0),
                        stop=(k == KD - 1),
                    )
                nc.vector.tensor_copy(
                    k_sb[:, m, PQ * n : PQ * (n + 1)], ps[:, :]
                )

            def o_group(lb, on_act=False):
                # out[l-block lb, :] = sum_e O^T[e, lb] @ Wo^T[e, :]
                # full-D groups: one 2-bank PSUM slab (same slab as scores),
                # stationary loaded once per e-block for both 512-col spans
                ps = psS.tile([128, D], f32, tag="sps", name=f"op{lb}")
                for k in range(MB):
                    for c in (0, 512):
                        nc.tensor.matmul(
                            ps[:, c : c + 512],
                            o_sb[:, k, 128 * lb : 128 * (lb + 1)],
                            wo_sb[:, k, c : c + 512],
                            start=(k == 0),
                            stop=(k == MB - 1),
                        )
                ot = kp.tile([128, D], bf16, tag="ot", bufs=2,
                             name=f"ot{lb}")
                if on_act:  # tail: exp is done, ACT is idle
                    nc.scalar.activation(ot[:, :], ps[:, :],
                                         mybir.ActivationFunctionType.Copy)
                else:
                    nc.vector.tensor_copy(ot[:, :], ps[:, :])
                nc.sync.dma_start(
                    out[128 * lb : 128 * (lb + 1), :],
                    ot[:, :],
                )

            # ---- attention head-chunk emitter ----
            # Blocks [i_lo, i_hi) of chunk j.  With partial_out, the raw
            # accumulator is staged to SBUF (bf16) and no divide happens;
            # with add_partial, a previously staged partial is added back
            # before the divide.
            def attn_head(h, j, i_lo=0, i_hi=None, partial_out=None,
                          add_partial=None):
                m = h // HPB
                poff = (h % HPB) * E
                nsb = SBC * (j + 1)
                if i_hi is None:
                    i_hi = nsb
                sps = [None] * i_hi
                pt = [None] * i_hi
                ops = psO.tile([128, CH], f32, tag="ops",
                               name=f"ops{h}_{j}_{i_lo}")

                def col0(i):
                    return max(0, 128 * i - CH * j)

                def _spans(c0):
                    # split [c0, CH) at 512-col PSUM bank boundaries
                    sp, c = [], c0
                    while c < CH:
                        n = min(512 - (c % 512), CH - c)
                        sp.append((c, n))
                        c += n
                    return sp

                def emit_scores(i):
                    c0 = col0(i)
                    sps[i] = psS.tile([128, CH], f32, tag="sps",
                                      name=f"sps{h}_{i}_{j}")
                    for (c, n) in _spans(c0):
                        nc.tensor.matmul(
                            sps[i][:, c : c + n],
                            k_sb[poff : poff + E, m, 128 * i : 128 * (i + 1)],
                            q_sb[poff : poff + E, m,
                                 CH * j + c : CH * j + c + n],
                            start=True,
                            stop=True,
                        )
                    pt[i] = kp.tile([128, CH], bf16, tag="p_t",
                                    name=f"pt{h}_{i}_{j}")
                    nc.scalar.activation(
                        pt[i][:, c0:CH],
                        sps[i][:, c0:CH],
                        mybir.ActivationFunctionType.Exp,
                        scale=scale,
                    )
                    if i >= SBC * j:  # diagonal band: causal mask
                        nc.vector.tensor_mul(
                            pt[i][:, c0 : c0 + 128],
                            pt[i][:, c0 : c0 + 128],
                            tri[:, :],
                        )

                def emit_av(i):
                    c0 = col0(i)
                    for (c, n) in _spans(c0):
                        # last contributing s-block for this 512-col span
                        be = min(CH, 512 * (c // 512) + 512)
                        i_last = min(i_hi - 1, (be + CH * j) // 128 - 1)
                        nc.tensor.matmul(
                            ops[0 : E + 1, c : c + n],
                            v_sb[:, i, h, :],
                            pt[i][:, c : c + n],
                            start=(i == i_lo),
                            stop=(i == i_last),
                        )

                emit_scores(i_lo)
                if i_lo + 1 < i_hi:
                    emit_scores(i_lo + 1)
                for i in range(i_lo + 2, i_hi):
                    emit_scores(i)
                    emit_av(i - 2)
                if i_lo + 1 < i_hi:
                    emit_av(i_hi - 2)
                emit_av(i_hi - 1)

                # stage O^T + denominator row to SBUF, freeing the PSUM
                # bank; chunk 0 stages on the (there) half-idle ACT engine
                o_raw = dp.tile([128, CH], f32, tag="o_raw", bufs=2)
                if j == 0:
                    nc.scalar.activation(
                        o_raw[0 : E + 1, :],
                        ops[0 : E + 1, :],
                        mybir.ActivationFunctionType.Copy,
                    )
                else:
                    nc.vector.tensor_copy(o_raw[0 : E + 1, :],
                                          ops[0 : E + 1, :])

                # divide chain, off the tensor critical path; the Vector
                # half is returned as a closure so the caller can defer it
                # behind the next head's mask/copy work
                rs0 = dp.tile([128, CH], f32, tag="rs0", bufs=2)
                nc.sync.dma_start(rs0[0:1, :], o_raw[E : E + 1, :])
                rr = dp.tile([128, CH], f32, tag="rr", bufs=2)
                nc.gpsimd.partition_broadcast(
                    rr[0:E, :], rs0[0:1, :], channels=E
                )

                def finish_chain():
                    nc.vector.reciprocal_approx_fast(rr[0:E, :], rr[0:E, :])
                    if poff == 0:
                        nc.vector.tensor_mul(
                            o_sb[0:E, m, CH * j : CH * (j + 1)],
                            o_raw[0:E, :],
                            rr[0:E, :],
                        )
                    else:
                        o_tmp = dp.tile([128, CH], bf16, tag="o_tmp", bufs=2)
                        nc.vector.tensor_mul(o_tmp[0:E, :], o_raw[0:E, :],
                                             rr[0:E, :])
                        nc.sync.dma_start(
                            o_sb[poff : poff + E, m, CH * j : CH * (j + 1)],
                            o_tmp[0:E, :],
                        )

                return finish_chain

            # ---- startup DMA: weights + first halves, unblocking order ----
            for k in range(KD):
                nc.sync.dma_start(wv_sb[:, k, :], wv[128 * k : 128 * (k + 1), :])
                _ld(vT, ipv, k, 0)
            for k in range(KD):
                nc.sync.dma_start(wq_sb[:, k, :], wq[128 * k : 128 * (k + 1), :])
                _ld(qT, ipq, k, 0)
            for k in range(KD):
                nc.sync.dma_start(wk_sb[:, k, :], wk[128 * k : 128 * (k + 1), :])
                _ld(kT, ipk, k, 0)
            for k in range(MB):
                nc.sync.dma_start(wo_sb[:, k, :], wo[128 * k : 128 * (k + 1), :])

            # ---- dense phase: first-half projections (l,s in [0, 1024)) ----
            for sb in range(SBC):
                v_group(sb)
            for m in range(MB):
                for n in range(2):
                    q_group(m, n)
            # second-half input loads chase the dense phase
            for k in range(KD):
                _ld(vT, ipv, k, 1)
            for m in range(MB):
                for n in range(2):
                    k_group(m, n)
            for k in range(KD):
                _ld(qT, ipq, k, 1)
                _ld(kT, ipk, k, 1)

            # ---- attention chunk 0 + second-half projections as fillers ----
            fillers = []
            for sb in range(SBC, 2 * SBC):
                fillers.append(lambda sb=sb: v_group(sb))
            for m in range(MB):
                for n in range(2, 4):
                    fillers.append(lambda m=m, n=n: q_group(m, n))
                    fillers.append(lambda m=m, n=n: k_group(m, n))
            nf, done = len(fillers), 0
            pending = None
            for h in range(HC):
                fin = attn_head(h, 0)
                if pending is not None:
                    pending()
                pending = fin
                upto = nf * (h + 1) // HC
                while done < upto:
                    fillers[done]()
                    done += 1
            pending()

            # ---- attention chunk 1 + chunk-0 output projection (4 of its
            # 16 groups held back to fill the post-AV divide-chain gap) ----
            fillers = [lambda lb=lb: o_group(lb) for lb in range(SBC)]
            reserve = fillers[-2:]
            fillers = fillers[:-2]
            nf, done = len(fillers), 0
            pending = None
            for h in range(HC):
                fin = attn_head(h, 1)
                if pending is not None:
                    pending()
                pending = fin
                upto = nf * (h + 1) // HC
                while done < upto:
                    fillers[done]()
                    done += 1
            pending()

            # ---- tail: reserved chunk-0 groups, then chunk-1 o-proj ----
            for f in reserve:
                f()
            for lb in range(SBC, 2 * SBC):
                o_group(lb, on_act=True)

            if dump:
                dq = nc.dram_tensor("dq", [128, MB, L], bf16, kind="ExternalOutput")
                dk = nc.dram_tensor("dk", [128, MB, S], bf16, kind="ExternalOutput")
                dv = nc.dram_tensor(
                    "dv", [128, S // 128, HC, E + 1], bf16, kind="ExternalOutput"
                )
                do = nc.dram_tensor("do", [128, MB, L], bf16, kind="ExternalOutput")
                nc.sync.dma_start(dq[:], q_sb[:])
                nc.sync.dma_start(dk[:], k_sb[:])
                nc.sync.dma_start(dv[:], v_sb[:])
                nc.sync.dma_start(do[:], o_sb[:])

    nc.compile()
    return nc


def _prep_inputs(queries, keys, values, Wq, bq, Wk, Wv, Wo):
    """Build the 8 per-core input maps (host-side shard + transpose + cast)."""
    MB = EC // 128
    in_maps = []
    qT = [np.ascontiguousarray(queries[b].T.astype(_BF16)) for b in range(B)]
    kT = [np.ascontiguousarray(keys[b].T.astype(_BF16)) for b in range(B)]
    vT = [np.ascontiguousarray(values[b].T.astype(_BF16)) for b in range(B)]
    wqs, wks, wvs, wos, bqs = [], [], [], [], []
    for g in range(GROUPS):
        sl = slice(g * EC, (g + 1) * EC)
        wqs.append(np.ascontiguousarray(Wq[sl, :].T.astype(_BF16)))
        wks.append(np.ascontiguousarray(Wk[sl, :].T.astype(_BF16)))
        wvs.append(np.ascontiguousarray(Wv[sl, :].T.astype(_BF16)))
        wos.append(np.ascontiguousarray(Wo[:, sl].T.astype(_BF16)))
        bqs.append(
            np.ascontiguousarray(
                bq[sl].astype(np.float32).reshape(MB, 128).T
            )
        )
    for c in range(NCORES):
        b, g = c // GROUPS, c % GROUPS
        in_maps.append(
            {
                "qT": qT[b], "kT": kT[b], "vT": vT[b],
                "wq": wqs[g], "wk": wks[g], "wv": wvs[g],
                "wo": wos[g], "bq": bqs[g],
            }
        )
    return in_maps


_NC_CACHE = {}


def kernel(queries, keys, values, attn_mask, Wq, bq, Wk, bk, Wv, bv, Wo, bo,
           _trace=False):
    from concourse.bass_utils import run_bass_kernel_spmd

    queries = np.asarray(queries, np.float32)
    keys = np.asarray(keys, np.float32)
    values = np.asarray(values, np.float32)
    Wq, Wk, Wv, Wo = (np.asarray(a, np.float32) for a in (Wq, Wk, Wv, Wo))
    bq, bk, bv, bo = (np.asarray(a, np.float32) for a in (bq, bk, bv, bo))

    if "nc" not in _NC_CACHE:
        _NC_CACHE["nc"] = build()
    nc = _NC_CACHE["nc"]

    in_maps = _prep_inputs(queries, keys, values, Wq, bq, Wk, Wv, Wo)
    res = run_bass_kernel_spmd(
        nc, in_maps, core_ids=list(range(NCORES)), trace=_trace
    )
    _NC_CACHE["last_results"] = res

    out = np.zeros((B, L, D), np.float32)
    for c in range(NCORES):
        out[c // GROUPS] += np.asarray(res.results[c]["out"], np.float32)
    # bv exits through the (row-sum-1) softmax as Wo @ bv; bo is direct.
    out += (Wo @ bv + bo)[None, None, :]
    return out
